# revision 2
# baseline (speedup 1.0000x reference)
"""Distributed crossbar-sim conv for nn_BasicConv2d_49632642072788.

Forward value of the reference is relu(_crossbar_conv(x, w)).  The crossbar
sim is a sum of 40 independent (channel-group g, activation-bit i) partial
units; each unit needs 5 convs (dummy + 4 weight bit-slices) and ADC
quantization whose min/max is global over THAT partial's full (N,Cout,H,W)
tensor.  We shard by unit: core c computes activation bit-plane c of all 5
channel groups (4x14ch + 1x8ch = 64 channel-convs per core -> perfectly
balanced), so every ADC min/max is core-local and only one final sum over
cores is needed.  Weight bit-slicing / activation bit-plane extraction are
cheap integer prep done on host; the 25 convs + ADC quant per core run on
the 8 NeuronCores via pmap (SPMD).  The device attempt runs in a
subprocess with a hard timeout; on failure the same math runs on CPU.
"""
import os
import sys
import subprocess
import tempfile
import numpy as np
import jax
import jax.numpy as jnp

WL_W, WL_A, CELL_BIT, SUBARRAY, ADC_BITS = 8, 8, 2, 128, 5
ONOFF = 10.0
KH = KW = 3
PAD = 1
CELL_RANGE = 2 ** CELL_BIT                 # 4
N_SLICES = WL_W // CELL_BIT                # 4
UPPER, LOWER = 1.0, 1.0 / ONOFF
GROUP_C = max(1, SUBARRAY // (KH * KW))    # 14
A_LEVELS = 2.0 ** WL_A - 1.0
W_LEVELS = 2.0 ** WL_W - 1.0

N, CIN, H, W_ = 8, 64, 56, 56
COUT = 128
NCORES = 8
GROUPS = [(g0, min(GROUP_C, CIN - g0)) for g0 in range(0, CIN, GROUP_C)]
NG = len(GROUPS)  # 5

DEVICE_TIMEOUT_S = int(os.environ.get("KERNEL_DEVICE_TIMEOUT_S", "420"))


def _conv(x, w):
    return jax.lax.conv_general_dilated(
        x, w, window_strides=(1, 1), padding=[(PAD, PAD), (PAD, PAD)],
        dimension_numbers=('NCHW', 'OIHW', 'NCHW'))


def _adc_q(x):
    mn, mx = jnp.min(x), jnp.max(x)
    step = (mx - mn) * (2.0 ** (-ADC_BITS))
    idx = jnp.clip(jnp.floor(jnp.where(step > 0, (x - mn) / step, 0.0)),
                   0.0, 2.0 ** ADC_BITS - 1.0)
    return mn + idx * step


def _core_fn(xb, wt, bitscale):
    # xb: (NG, N, GROUP_C, H, W)  bit-plane activations (zero-padded chans)
    # wt: (NG, 1+N_SLICES, COUT, GROUP_C, KH, KW)  dummy + slice conductances
    # bitscale: (1,) = 2**bit for this core
    acc = jnp.zeros((N, COUT, H, W_), jnp.float32)
    for j in range(NG):
        d = _adc_q(_conv(xb[j], wt[j, 0]))
        for k in range(N_SLICES):
            p = _adc_q(_conv(xb[j], wt[j, 1 + k]))
            acc = acc + (p - d) * float(CELL_RANGE ** k)
    return acc * bitscale[0]


def _host_prep(x, w):
    # WAGE weight quantization -> integer weight levels
    delta = 2.0 ** (1 - WL_W)
    wq = np.clip(np.round(np.asarray(w, np.float64) / delta) * delta,
                 -1.0 + delta, 1.0 - delta)
    w_int = np.round((wq + 1.0) * 0.5 * W_LEVELS)          # [0, 255]
    a_int = np.round(np.clip(np.asarray(x, np.float64), 0.0, 1.0)
                     * A_LEVELS).astype(np.int64)          # [0, 255]

    xb_all = np.zeros((NCORES, NG, N, GROUP_C, H, W_), np.float32)
    wt_all = np.zeros((NCORES, NG, 1 + N_SLICES, COUT, GROUP_C, KH, KW),
                      np.float32)
    for c in range(NCORES):
        bit = c
        for j, (g0, cg) in enumerate(GROUPS):
            xb_all[c, j, :, :cg] = ((a_int[:, g0:g0 + cg] >> bit) & 1)
            wg = w_int[:, g0:g0 + cg]
            xdec = wg.copy()
            for k in range(N_SLICES):
                rem = np.mod(xdec, CELL_RANGE)
                xdec = np.floor(xdec / CELL_RANGE)
                wt_all[c, j, 1 + k, :, :cg] = \
                    (UPPER - LOWER) * rem + (CELL_RANGE - 1) * LOWER
            wt_all[c, j, 0, :, :cg] = \
                (CELL_RANGE - 1) * (UPPER + LOWER) / 2.0
    bitscale = (2.0 ** np.arange(NCORES, dtype=np.float32)).reshape(NCORES, 1)
    return xb_all, wt_all, bitscale


def _finish(parts):
    out = parts.sum(axis=0)
    out = out / (UPPER - LOWER) * 2.0 / (W_LEVELS * A_LEVELS)
    return np.maximum(out, 0.0).astype(np.float32)


def _device_child(tmpdir):
    """Runs in a subprocess: execute the SPMD program on the 8 NeuronCores."""
    xb = np.load(os.path.join(tmpdir, "xb.npy"))
    wt = np.load(os.path.join(tmpdir, "wt.npy"))
    bs = np.load(os.path.join(tmpdir, "bs.npy"))
    parts = np.asarray(jax.pmap(_core_fn)(xb, wt, bs))
    np.save(os.path.join(tmpdir, "parts.npy"), parts)


def _run_device(xb_all, wt_all, bitscale):
    tmpdir = tempfile.mkdtemp()
    np.save(os.path.join(tmpdir, "xb.npy"), xb_all)
    np.save(os.path.join(tmpdir, "wt.npy"), wt_all)
    np.save(os.path.join(tmpdir, "bs.npy"), bitscale)
    code = ("import sys; sys.path.insert(0, %r); "
            "import kernel; kernel._device_child(%r)"
            % (os.path.dirname(os.path.abspath(__file__)), tmpdir))
    subprocess.run([sys.executable, "-c", code], timeout=DEVICE_TIMEOUT_S,
                   check=True, stdout=subprocess.DEVNULL,
                   stderr=subprocess.DEVNULL)
    return np.load(os.path.join(tmpdir, "parts.npy"))


def _run_cpu(xb_all, wt_all, bitscale):
    cpu_fn = jax.jit(_core_fn, backend="cpu")
    cpu = jax.devices("cpu")[0]
    parts = [np.asarray(cpu_fn(jax.device_put(xb_all[c], cpu),
                               jax.device_put(wt_all[c], cpu),
                               jax.device_put(bitscale[c], cpu)))
             for c in range(NCORES)]
    return np.stack(parts)


def kernel(x, w):
    xb_all, wt_all, bitscale = _host_prep(x, w)
    try:
        parts = _run_device(xb_all, wt_all, bitscale)
    except Exception:
        parts = _run_cpu(xb_all, wt_all, bitscale)
    return _finish(parts)
